# revision 24
# baseline (speedup 1.0000x reference)
"""Distributed multi-head attention kernel for 8 TRN2 NeuronCores.

Problem: B=2, S=2048, H=1024 (16 heads x 64), fp32 in/out.
Sharding: core c = 4*b + g handles batch b and head-group g (4 heads, 256
hidden cols). Wq/Wk/Wv column-sharded, Wo row-sharded; a per-q-chunk
bf16 ReduceScatter over each 4-core batch group yields each core's
4x128-row slices of the output (overlapped with compute).

Dataflow per core (transpose-free attention, bf16 matmuls, fp32 PSUM):
  x^T via DMA-XBAR transpose (host pre-casts x, W to bf16)
  Q^T,K^T = (W^T x^T) in [j,t] layout; V = x^T-stationary @ Wv
  scores^T[k,q] = K^T.T@Q^T with two heads packed into PE row groups;
  Pt = exp(scores/8), no max subtraction (scores ~ N(0,1), exact softmax)
  ctx^T[d,q] (+ sums row via ones column in V) = [V|1].T @ Pt
  normalize via K=1 broadcast matmul of 1/sums
  out partial[t,o] = ctx^T-stationary @ Wo -> bf16 -> chunked ReduceScatter.
bq/bk applied on-device (ACT bias); bv/bo folded in on host (exact:
out += bv@Wo + bo, since softmax rows sum to one).
"""

import sys

for p in ("/opt/trn_rl_repo",):
    if p not in sys.path:
        sys.path.insert(0, p)

from contextlib import ExitStack

import ml_dtypes
import numpy as np

from concourse import bacc, mybir, tile
from concourse.bass import ds
from concourse.bass_utils import run_bass_kernel_spmd

F32 = mybir.dt.float32
BF16 = mybir.dt.bfloat16
AF = mybir.ActivationFunctionType

B, S, H = 2, 2048, 1024
NH, D = 16, 64
NCORES = 8
GROUPS = [[0, 1, 2, 3], [4, 5, 6, 7]]
JG = 256           # hidden cols per core (4 heads)
SO = S // 4        # 512 output rows per core after reduce-scatter

_cache = {}


def _build():
    nc = bacc.Bacc("TRN2", target_bir_lowering=False, debug=False,
                   num_devices=NCORES)
    x_d = nc.dram_tensor("xbf", [S, H], BF16, kind="ExternalInput")
    wq_d = nc.dram_tensor("wq", [H, JG], BF16, kind="ExternalInput")
    wk_d = nc.dram_tensor("wk", [H, JG], BF16, kind="ExternalInput")
    wv_d = nc.dram_tensor("wv", [H, JG], BF16, kind="ExternalInput")
    wo_d = nc.dram_tensor("wo", [JG, H], BF16, kind="ExternalInput")
    bq_d = nc.dram_tensor("bqc", [128, 2], F32, kind="ExternalInput")
    bk_d = nc.dram_tensor("bkc", [128, 2], F32, kind="ExternalInput")
    out_d = nc.dram_tensor("out", [SO, H], BF16, kind="ExternalOutput")

    def mm(ps, lhsT, rhs, start, stop, tile_position=None):
        nc.tensor.matmul(ps, lhsT, rhs, start=start, stop=stop,
                         tile_position=tile_position)

    with tile.TileContext(nc) as tc, ExitStack() as st:
        consts = st.enter_context(tc.tile_pool(name="consts", bufs=1))
        ones1 = consts.tile([1, 64], BF16)
        nc.vector.memset(ones1[:], 1.0)
        bq_sb = consts.tile([128, 2], F32)
        nc.gpsimd.dma_start(bq_sb[:], bq_d[:, :])
        bk_sb = consts.tile([128, 2], F32)
        nc.gpsimd.dma_start(bk_sb[:], bk_d[:, :])

        wpool = st.enter_context(tc.tile_pool(name="weights", bufs=1))
        # w*_sb[:, 256*s + j] = W[s*128 + p, j]  (k-slice-major free layout)
        w_sb = {}
        for wname, wd, inner in (("wq", wq_d, 256), ("wk", wk_d, 256),
                                 ("wv", wv_d, 256), ("wo", wo_d, 1024)):
            wt = wpool.tile([128, 2048], BF16, name=f"{wname}sb",
                            tag=f"{wname}sb")
            nc.gpsimd.dma_start(
                wt[:].rearrange("p (s j) -> p s j", j=inner),
                wd.ap().rearrange("(s p) j -> p s j", p=128))
            w_sb[wname] = wt
        wq_sb, wk_sb, wv_sb, wo_sb = (w_sb[n] for n in ("wq", "wk", "wv", "wo"))

        qkv = st.enter_context(tc.tile_pool(name="qkv", bufs=1))
        qT = [qkv.tile([128, S], BF16, name=f"qT{j}", tag=f"qT{j}")
              for j in range(2)]
        kT = [qkv.tile([128, S], BF16, name=f"kT{j}", tag=f"kT{j}")
              for j in range(2)]
        ctxT = [qkv.tile([128, S], BF16, name=f"cT{j}", tag=f"cT{j}")
                for j in range(2)]
        # V padded per head with a ones column: head h at cols 65h..65h+63
        v_sb = [qkv.tile([128, 260], BF16, name=f"v{i}", tag=f"v{i}")
                for i in range(16)]

        # ---- Phase A: x^T via DMA XBAR transpose ----
        xTp = st.enter_context(tc.tile_pool(name="xT", bufs=1))
        xT = [xTp.tile([128, S], BF16, name=f"xT{s}", tag=f"xT{s}")
              for s in range(8)]
        for tq in range(4):
            for s in range(8):
                nc.sync.dma_start(xT[s][:, ds(512 * tq, 512)],
                                  x_d.ap()[ds(512 * tq, 512), ds(128 * s, 128)],
                                  transpose=True)

        # ---- Phase B: projections ----
        with tc.tile_pool(name="pps", bufs=4, space="PSUM") as pps:
            for jt in range(2):
                for tq in range(4):
                    ps = pps.tile([128, 512], F32, tag="ps")
                    for s in range(8):
                        mm(ps[:], wq_sb[:, ds(256 * s + 128 * jt, 128)],
                           xT[s][:, ds(512 * tq, 512)], s == 0, s == 7)
                    nc.scalar.activation(qT[jt][:, ds(512 * tq, 512)], ps[:],
                                         AF.Identity, bias=bq_sb[:, ds(jt, 1)])
                    ps = pps.tile([128, 512], F32, tag="ps")
                    for s in range(8):
                        mm(ps[:], wk_sb[:, ds(256 * s + 128 * jt, 128)],
                           xT[s][:, ds(512 * tq, 512)], s == 0, s == 7)
                    nc.scalar.activation(kT[jt][:, ds(512 * tq, 512)], ps[:],
                                         AF.Identity, bias=bk_sb[:, ds(jt, 1)])

            for tv in range(16):
                ps = pps.tile([128, 512], F32, tag="ps")
                for s in range(8):
                    mm(ps[:, 0:256], xT[s][:, ds(128 * tv, 128)],
                       wv_sb[:, ds(256 * s, 256)], s == 0, s == 7)
                nc.vector.memset(v_sb[tv][:], 1.0)
                nc.vector.tensor_copy(
                    v_sb[tv][:].rearrange("p (h c) -> p h c", c=65)[:, :, 0:64],
                    ps[:, 0:256].rearrange("p (h c) -> p h c", c=64))

        # ---- Phase C/D/E: attention + out-proj + chunked reduce-scatter ----
        dram = st.enter_context(tc.tile_pool(name="dram", bufs=1, space="DRAM"))
        partial_c = [dram.tile([512, H], BF16, name=f"pc{i}", tag=f"pc{i}")
                     for i in range(4)]
        rs_c = [dram.tile([128, H], BF16, name=f"rc{i}", tag=f"rc{i}")
                for i in range(4)]
        # the last chunk's RS is fully exposed at the kernel tail; split it
        # in half so the first half overlaps the remaining out-projection
        ph_c = [dram.tile([256, H], BF16, name=f"ph{i}", tag=f"ph{i}")
                for i in range(2)]
        rh_c = [dram.tile([64, H], BF16, name=f"rh{i}", tag=f"rh{i}")
                for i in range(2)]

        with tc.tile_pool(name="scps", bufs=2, space="PSUM") as scps, \
             tc.tile_pool(name="ctxps", bufs=2, space="PSUM") as ctxps, \
             tc.tile_pool(name="bcps", bufs=1, space="PSUM") as bcps, \
             tc.tile_pool(name="psb", bufs=3) as psb, \
             tc.tile_pool(name="rsb", bufs=2) as rsb, \
             tc.tile_pool(name="osb", bufs=4) as osb:
            for tq in range(4):
                # attention for all 4 heads on this 512-token q slice,
                # head pairs packed into PE row-groups (K=64 each)
                cus = []
                for hp in range(2):
                    cA = ctxps.tile([65, 512], F32, tag="cps")
                    cB = ctxps.tile([65, 512], F32, tag="cps")
                    for kt in range(16):
                        sp = scps.tile([128, 1024], F32, tag="sps")
                        mm(sp[:, 0:512],
                           kT[hp][0:64, ds(128 * kt, 128)],
                           qT[hp][0:64, ds(512 * tq, 512)],
                           True, True, tile_position=(0, 0))
                        mm(sp[:, 512:1024],
                           kT[hp][64:128, ds(128 * kt, 128)],
                           qT[hp][64:128, ds(512 * tq, 512)],
                           True, True, tile_position=(64, 0))
                        pt = psb.tile([128, 1024], BF16, tag="pt")
                        nc.scalar.activation(pt[:], sp[:], AF.Exp, scale=0.125)
                        mm(cA[:], v_sb[kt][:, ds(65 * (2 * hp), 65)],
                           pt[:, 0:512], kt == 0, kt == 15)
                        mm(cB[:], v_sb[kt][:, ds(65 * (2 * hp + 1), 65)],
                           pt[:, 512:1024], kt == 0, kt == 15)
                    for h, cps in ((2 * hp, cA), (2 * hp + 1, cB)):
                        # evacuate PSUM right away (frees the slot for the
                        # next pair); cast the sums row for the broadcast mm
                        cu = rsb.tile([65, 512], F32, tag="cu", bufs=4)
                        nc.vector.tensor_copy(cu[:], cps[:])
                        sm16 = rsb.tile([1, 512], BF16, tag="sm16", bufs=4)
                        nc.vector.tensor_copy(sm16[:], cu[ds(64, 1), :])
                        cus.append((h, cu, sm16))
                # normalize all 4 heads: broadcast the raw sums with a K=1
                # matmul, then a WIDE reciprocal (64 lanes, not 1) + multiply
                for h, cu, sm16 in cus:
                    hp, po = h // 2, 64 * (h % 2)
                    bc = bcps.tile([64, 512], F32, tag="bc")
                    mm(bc[:], ones1[:], sm16[:], True, True)
                    rbc = rsb.tile([64, 512], F32, tag="rbc")
                    nc.vector.reciprocal_approx_fast(rbc[:], bc[:])
                    nc.vector.tensor_mul(
                        ctxT[hp][ds(po, 64), ds(512 * tq, 512)],
                        cu[0:64, :], rbc[:])

                # output projection for this 512-token slice -> partial
                for tt in range(4 * tq, 4 * tq + 4):
                    ps = scps.tile([128, 1024], F32, tag="sps")
                    for oo in range(2):
                        for js in range(2):
                            mm(ps[:, ds(512 * oo, 512)],
                               ctxT[js][:, ds(128 * tt, 128)],
                               wo_sb[:, ds(1024 * js + 512 * oo, 512)],
                               js == 0, js == 1)
                    ot = osb.tile([128, 1024], BF16, tag="ot")
                    nc.vector.tensor_copy(ot[:], ps[:])
                    tl = tt - 4 * tq
                    if tq < 3:
                        nc.sync.dma_start(
                            partial_c[tq][ds(128 * tl, 128), :], ot[:])
                    else:
                        nc.sync.dma_start(
                            ph_c[tl // 2][ds(128 * (tl % 2), 128), :], ot[:])
                        if tl % 2 == 1:
                            h = tl // 2
                            nc.gpsimd.collective_compute(
                                "ReduceScatter", mybir.AluOpType.add,
                                replica_groups=GROUPS,
                                ins=[ph_c[h].opt()], outs=[rh_c[h].opt()])
                            nc.sync.dma_start(
                                out_d[ds(384 + 64 * h, 64), :], rh_c[h][:])

                # reduce-scatter this chunk over the batch group; the
                # final chunk goes as two halves so its first half overlaps
                # the tail of the out-projection
                if tq < 3:
                    nc.gpsimd.collective_compute(
                        "ReduceScatter", mybir.AluOpType.add,
                        replica_groups=GROUPS,
                        ins=[partial_c[tq].opt()], outs=[rs_c[tq].opt()])
                    nc.sync.dma_start(out_d[ds(128 * tq, 128), :],
                                      rs_c[tq][:])

    nc.compile()
    return nc


def _get_nc():
    if "nc" not in _cache:
        _cache["nc"] = _build()
    return _cache["nc"]


def _in_maps(x, Wq, bq, Wk, bk, Wv, bv, Wo, bo):
    bf = ml_dtypes.bfloat16
    maps = []
    for c in range(NCORES):
        b, g = c // 4, c % 4
        j0 = JG * g
        maps.append({
            "xbf": np.ascontiguousarray(x[b]).astype(bf),
            "wq": np.ascontiguousarray(Wq[:, j0:j0 + JG]).astype(bf),
            "wk": np.ascontiguousarray(Wk[:, j0:j0 + JG]).astype(bf),
            "wv": np.ascontiguousarray(Wv[:, j0:j0 + JG]).astype(bf),
            "wo": np.ascontiguousarray(Wo[j0:j0 + JG, :]).astype(bf),
            "bqc": np.ascontiguousarray(bq[j0:j0 + JG].reshape(2, 128).T),
            "bkc": np.ascontiguousarray(bk[j0:j0 + JG].reshape(2, 128).T),
        })
    return maps


def kernel(x, Wq, bq, Wk, bk, Wv, bv, Wo, bo, _trace=False):
    x, Wq, bq, Wk, bk, Wv, bv, Wo, bo = (
        np.asarray(a, dtype=np.float32)
        for a in (x, Wq, bq, Wk, bk, Wv, bv, Wo, bo))
    nc = _get_nc()
    res = run_bass_kernel_spmd(nc, _in_maps(x, Wq, bq, Wk, bk, Wv, bv, Wo, bo),
                               core_ids=list(range(NCORES)), trace=_trace)
    out = np.empty((B, S, H), np.float32)
    for c in range(NCORES):
        b, g = c // 4, c % 4
        oc = np.asarray(res.results[c]["out"], dtype=np.float32)
        for tq in range(3):
            out[b, 512 * tq + 128 * g:512 * tq + 128 * (g + 1), :] = \
                oc[128 * tq:128 * (tq + 1)]
        for h in range(2):
            r0 = 1536 + 256 * h + 64 * g
            out[b, r0:r0 + 64, :] = oc[384 + 64 * h:384 + 64 * (h + 1)]
    out += bv @ Wo + bo  # exact: softmax rows sum to 1
    if _trace:
        return out, res
    return out
